# revision 16
# baseline (speedup 1.0000x reference)
"""Trainium2 Bass kernel for CausalSelfAttention (GQA + qk-rmsnorm + rope + head gating).

Sharding: 8 cores = 2 (batch) x 4 (kv-head groups). Each core computes the
full attention for one batch element and one kv-head group (4 q heads), plus
its slice of the output projection; partial projection outputs are summed on
the host (bf16 partials, fp32 accumulation).

Per-core pipeline, merged across phases per 4-token-tile group g:
  A) fused QKV+gate projection for tiles 4g..4g+3 -> bf16 cast ->
     full-width rope (3 DVE ops/group, swap-AP + pre-signed sin table) ->
     rms stats (mul+reduce) -> scale -> DMA-transpose q,k to head-dim-major
  B) causal attention for q chunk g in S^T layout, software-pipelined:
     S^T = K @ Q^T (PE), P = exp(S/sqrt(d)) (ACT), diagonal mask (DVE),
     Y = P @ [V | 1] (PE, ones column = softmax denominator), with the
     previous chunk's output-projection matmuls interleaved into the
     tensor queue to fill exp-latency stalls
  C) output projection partial in bf16, 256KB stores
k-side elementwise work (rope/stats/scale/v-copy) runs on GpSimd to keep
the DVE under the tensor-engine roofline.
"""

import numpy as np
import ml_dtypes
from contextlib import ExitStack

import concourse.bass as bass
import concourse.bacc as bacc
import concourse.mybir as mybir
import concourse.tile as tile
from concourse.bass_utils import run_bass_kernel_spmd

BF16 = mybir.dt.bfloat16
F32 = mybir.dt.float32
NPBF = ml_dtypes.bfloat16

B, T, D = 2, 2048, 2048
H, HKV, HD = 16, 4, 128
HALF = HD // 2
NHEAD = H // HKV          # q heads per core (group)
NT = T // 128             # 16 token tiles
NG = 4                    # 4-tile groups
NCHUNK = D // 128         # 16 contraction chunks
NKVG = HD + HD + NHEAD    # 128 k + 128 v + 4 gate = 260
ROPE_BASE = 10000.0
EPS = float(np.finfo(np.float32).eps)
SM_SCALE = 1.0 / float(np.sqrt(HD))

_CACHE = {}


def _build_program():
    nc = bacc.Bacc("TRN2", target_bir_lowering=False, debug=False,
                   enable_asserts=False, num_devices=8)

    xT_d = nc.dram_tensor("xT", [D, T], BF16, kind="ExternalInput").ap()
    wq_d = nc.dram_tensor("wq", [D, NHEAD * HD], BF16, kind="ExternalInput").ap()
    wkvg_d = nc.dram_tensor("wkvg", [D, NKVG], BF16, kind="ExternalInput").ap()
    wproj_d = nc.dram_tensor("wproj", [NHEAD * HD, D], BF16, kind="ExternalInput").ap()
    cos_d = nc.dram_tensor("cosd", [T, HD], BF16, kind="ExternalInput").ap()
    sin_d = nc.dram_tensor("sind", [T, HD], BF16, kind="ExternalInput").ap()
    qgain_d = nc.dram_tensor("qgain", [1, NHEAD], F32, kind="ExternalInput").ap()
    gateb_d = nc.dram_tensor("gateb", [1, NHEAD], F32, kind="ExternalInput").ap()
    mask_d = nc.dram_tensor("masks", [128, 128], BF16, kind="ExternalInput").ap()
    out_d = nc.dram_tensor("out", [T, D], BF16, kind="ExternalOutput").ap()

    AF = mybir.ActivationFunctionType

    with tile.TileContext(nc) as tc, ExitStack() as ctx:
        consts = ctx.enter_context(tc.tile_pool(name="consts", bufs=1))

        # ---- resident tensors (loads interleaved so compute starts early) ----
        xT_sb = consts.tile([128, NCHUNK, T], BF16)
        wq_sb = consts.tile([128, NCHUNK, NHEAD * HD], BF16)
        wkvg_sb = consts.tile([128, NCHUNK, NKVG], BF16)
        for c in range(NCHUNK):
            cs = slice(c * 128, (c + 1) * 128)
            nc.sync.dma_start(out=xT_sb[:, c, :], in_=xT_d[cs, :])
            nc.sync.dma_start(out=wq_sb[:, c, :], in_=wq_d[cs, :])
            nc.sync.dma_start(out=wkvg_sb[:, c, :], in_=wkvg_d[cs, :])
        wproj_sb = consts.tile([128, NHEAD, D], BF16)
        for h in range(NHEAD):
            nc.scalar.dma_start(out=wproj_sb[:, h, :],
                                in_=wproj_d[h * 128:(h + 1) * 128, :])
        cos_sb = consts.tile([128, NT, HD], BF16)
        nc.scalar.dma_start(out=cos_sb,
                            in_=cos_d.rearrange("(tt p) i -> p tt i", p=128))
        sin_sb = consts.tile([128, NT, HD], BF16)
        nc.scalar.dma_start(out=sin_sb,
                            in_=sin_d.rearrange("(tt p) i -> p tt i", p=128))
        qgain_sb = consts.tile([128, NHEAD], F32)
        nc.scalar.dma_start(out=qgain_sb, in_=bass.AP(
            tensor=qgain_d.tensor, offset=qgain_d.offset,
            ap=[[0, 128], [1, NHEAD]]))
        gateb_sb = consts.tile([128, NHEAD], F32)
        nc.scalar.dma_start(out=gateb_sb, in_=bass.AP(
            tensor=gateb_d.tensor, offset=gateb_d.offset,
            ap=[[0, 128], [1, NHEAD]]))
        mask_sb = consts.tile([128, 128], BF16)
        nc.scalar.dma_start(out=mask_sb, in_=mask_d)

        qT_sb = consts.tile([128, NHEAD, T], BF16)   # head-dim-major q
        kT_sb = consts.tile([128, T], BF16)          # head-dim-major k
        v_sb = consts.tile([128, NT, HD + 1], BF16)  # [v | ones] per ki tile
        nc.vector.memset(v_sb[:, :, HD:HD + 1], 1.0)
        yT_sb = consts.tile([128, NHEAD, T], BF16)   # head-dim-major gated y
        gate_sb = consts.tile([128, NT, NHEAD], F32)
        eps_sb = consts.tile([128, 1], F32)
        nc.vector.memset(eps_sb, EPS)

        a_sb = ctx.enter_context(tc.tile_pool(name="phA", bufs=2))
        b_sb = ctx.enter_context(tc.tile_pool(name="phB", bufs=3))
        ps = ctx.enter_context(tc.tile_pool(name="ps", bufs=1, space="PSUM"))

        def swap_halves(ap3):
            """[p, n, HD] AP -> same with the two HD/2 halves swapped."""
            return bass.AP(tensor=ap3.tensor, offset=ap3.offset + HALF,
                           ap=[ap3.ap[0], ap3.ap[1], [-HALF, 2], [1, HALF]])

        def split_halves(ap3):
            """[p, n, HD] AP -> [p, n, 2, HD/2] (no swap), to match shapes."""
            return bass.AP(tensor=ap3.tensor, offset=ap3.offset,
                           ap=[ap3.ap[0], ap3.ap[1], [HALF, 2], [1, HALF]])

        # ---------------- proj thunk generator (phase C) ----------------
        def make_proj(qc):
            def gen():
                for half in range(2):          # nch pairs (0,1) and (2,3)
                    for qs in range(4):
                        tt = qc * 4 + qs
                        ts = slice(tt * 128, (tt + 1) * 128)
                        o_st = b_sb.tile([128, 1024], BF16, tag="o_st", bufs=2)
                        for sub in range(2):
                            nch = half * 2 + sub
                            o_ps = ps.tile([128, 512], F32, tag="qkv",
                                           bufs=2)
                            for h in range(NHEAD):
                                yield lambda o_ps=o_ps, h=h, ts=ts, nch=nch: \
                                    nc.tensor.matmul(
                                        o_ps, lhsT=yT_sb[:, h, ts],
                                        rhs=wproj_sb[:, h,
                                                     nch * 512:(nch + 1) * 512],
                                        start=(h == 0), stop=(h == NHEAD - 1))
                            if sub == 0:
                                yield lambda o_ps=o_ps, o_st=o_st: \
                                    nc.scalar.activation(
                                        out=o_st[:, 0:512], in_=o_ps,
                                        func=AF.Copy)
                            else:
                                yield lambda o_ps=o_ps, o_st=o_st: \
                                    nc.vector.tensor_copy(
                                        out=o_st[:, 512:1024], in_=o_ps)
                        yield lambda o_st=o_st, ts=ts, half=half: \
                            nc.sync.dma_start(
                                out=out_d[ts, half * 1024:(half + 1) * 1024],
                                in_=o_st)
            return gen()

        def drain(gen, n):
            if gen is None:
                return
            for _ in range(n):
                try:
                    next(gen)()
                except StopIteration:
                    return

        proj_gen = None

        for g in range(NG):
            # ================= Phase A: tiles 4g .. 4g+3 =================
            qa_g = a_sb.tile([128, 4, NHEAD * HD], BF16, tag="qa_g")
            kb_g = a_sb.tile([128, 4, NKVG], BF16, tag="kb_g")
            qst_g = a_sb.tile([128, 4, NHEAD, HD], BF16, tag="qst_g")
            kst_g = a_sb.tile([128, 4, HD], BF16, tag="kst_g")
            uk_g = a_sb.tile([128, 4, HD], BF16, tag="uk_g")
            sq_g = a_sb.tile([128, 4, NHEAD * HD], BF16, tag="sq_g")
            sqk_g = a_sb.tile([128, 4, HD], BF16, tag="sqk_g")
            glog_g = a_sb.tile([128, 4, NHEAD], F32, tag="glog_g")
            msq_g = a_sb.tile([128, 4, NHEAD + 1], F32, tag="msq_g")
            rtmp_g = a_sb.tile([128, 4, NHEAD + 1], F32, tag="rtmp_g")
            rinv_g = a_sb.tile([128, 4, NHEAD + 1], F32, tag="rinv_g")
            rq_g = a_sb.tile([128, 4, NHEAD], F32, tag="rq_g")

            for ti in range(4):
                tt = g * 4 + ti
                ts = slice(tt * 128, (tt + 1) * 128)
                q_ps = ps.tile([128, 512], F32, tag="qkv", bufs=2)
                for c in range(NCHUNK):
                    nc.tensor.matmul(q_ps, lhsT=xT_sb[:, c, ts],
                                     rhs=wq_sb[:, c, :],
                                     start=(c == 0), stop=(c == NCHUNK - 1))
                nc.vector.tensor_copy(out=qa_g[:, ti, :], in_=q_ps)
                b_ps = ps.tile([128, 512], F32, tag="qkv", bufs=2)
                for c in range(NCHUNK):
                    nc.tensor.matmul(b_ps[:, 0:NKVG], lhsT=xT_sb[:, c, ts],
                                     rhs=wkvg_sb[:, c, :],
                                     start=(c == 0), stop=(c == NCHUNK - 1))
                nc.vector.tensor_copy(out=kb_g[:, ti, :], in_=b_ps[:, 0:NKVG])
                nc.vector.tensor_copy(out=v_sb[:, tt, 0:HD],
                                      in_=kb_g[:, ti, HD:2 * HD])

            # gate logits (+bias broadcast over the 4 tiles)
            gateb_b = bass.AP(tensor=gateb_sb.tensor, offset=gateb_sb.offset,
                              ap=[gateb_sb.ap[0], [0, 4], [1, NHEAD]])
            nc.gpsimd.tensor_add(glog_g, kb_g[:, :, 2 * HD:2 * HD + NHEAD],
                                 gateb_b)

            # ---- q rope per tile: full-width pre-signed tables, h-bcast ----
            for ti in range(4):
                tt = g * 4 + ti
                u_t = a_sb.tile([128, NHEAD, HD], BF16, tag="u_t", bufs=2)
                qa_t = qa_g[:, ti, :]
                qa_h = bass.AP(tensor=qa_t.tensor, offset=qa_t.offset,
                               ap=[qa_t.ap[0], [HD, NHEAD], [1, HD]])
                qst_t = qst_g[:, ti, :, :]
                cos_t = cos_sb[:, tt, :]
                cos_h = bass.AP(tensor=cos_t.tensor, offset=cos_t.offset,
                                ap=[cos_t.ap[0], [0, NHEAD], [1, HD]])
                sin_t = sin_sb[:, tt, :]
                sin_h = bass.AP(tensor=sin_t.tensor, offset=sin_t.offset,
                                ap=[sin_t.ap[0], [0, NHEAD], [1, HD]])
                nc.vector.tensor_mul(qst_t, qa_h, cos_h)
                nc.vector.tensor_mul(split_halves(u_t), swap_halves(qa_h),
                                     split_halves(sin_h))
                nc.vector.tensor_add(qst_t, qst_t, u_t)

            # ---- k rope on gpsimd ----
            cos_t = cos_sb[:, g * 4:(g + 1) * 4, :]
            sin_t = sin_sb[:, g * 4:(g + 1) * 4, :]
            kin = kb_g[:, :, 0:HD]
            nc.gpsimd.tensor_mul(kst_g, kin, cos_t)
            nc.gpsimd.tensor_mul(split_halves(uk_g), swap_halves(kin),
                                 split_halves(sin_t))
            nc.gpsimd.tensor_add(kst_g, kst_g, uk_g)

            # ---- mean-square (rope preserves norms; use rotated values) ----
            nc.vector.tensor_mul(sq_g, qst_g, qst_g)
            msq_q = bass.AP(tensor=msq_g.tensor, offset=msq_g.offset,
                            ap=[msq_g.ap[0], [NHEAD + 1, 4], [1, NHEAD]])
            sq_red = bass.AP(tensor=sq_g.tensor, offset=sq_g.offset,
                             ap=[sq_g.ap[0], [HD, 16], [1, HD]])
            nc.vector.tensor_reduce(msq_q, sq_red,
                                    axis=mybir.AxisListType.X,
                                    op=mybir.AluOpType.add)
            nc.gpsimd.tensor_mul(sqk_g, kst_g, kst_g)
            nc.vector.tensor_reduce(msq_g[:, :, NHEAD:NHEAD + 1], sqk_g,
                                    axis=mybir.AxisListType.X,
                                    op=mybir.AluOpType.add)

            # ---- batched scalar math (Exp/Ln only: one ACT table set) ----
            # gate = 1/(1+exp(-glog)); rinv = exp(-0.5*ln(msq/HD + eps))
            gexp_g = a_sb.tile([128, 4, NHEAD], F32, tag="gexp_g")
            nc.scalar.activation(
                out=gexp_g.rearrange("p a b -> p (a b)"),
                in_=glog_g.rearrange("p a b -> p (a b)"), func=AF.Exp,
                scale=-1.0)
            gden_g = a_sb.tile([128, 4, NHEAD], F32, tag="gden_g")
            nc.vector.tensor_scalar_add(gden_g, gexp_g, 1.0)
            nc.vector.reciprocal(gate_sb[:, g * 4:(g + 1) * 4, :], gden_g)
            nc.scalar.activation(out=rtmp_g, in_=msq_g, func=AF.Ln,
                                 scale=1.0 / HD, bias=eps_sb)
            nc.scalar.activation(out=rinv_g, in_=rtmp_g, func=AF.Exp,
                                 scale=-0.5)
            qgain_b = bass.AP(tensor=qgain_sb.tensor, offset=qgain_sb.offset,
                              ap=[qgain_sb.ap[0], [0, 4], [1, NHEAD]])
            nc.vector.tensor_mul(rq_g, rinv_g[:, :, 0:NHEAD], qgain_b)

            # ---- scale into fresh staging tiles + transpose ----
            kfin_g = a_sb.tile([128, 4, HD], BF16, tag="kfin_g")
            rk_b = bass.AP(tensor=rinv_g.tensor,
                           offset=rinv_g.offset + NHEAD,
                           ap=[rinv_g.ap[0], [NHEAD + 1, 4], [0, HD]])
            nc.vector.tensor_mul(kfin_g, kst_g, rk_b)
            for ti in range(4):
                tt = g * 4 + ti
                ts = slice(tt * 128, (tt + 1) * 128)
                q_fin = a_sb.tile([128, NHEAD, HD], BF16, tag="q_fin",
                                  bufs=2)
                rq_t = rq_g[:, ti, :]
                rq_b = bass.AP(tensor=rq_t.tensor, offset=rq_t.offset,
                               ap=[rq_t.ap[0], [1, NHEAD], [0, HD]])
                nc.vector.tensor_mul(q_fin, qst_g[:, ti, :, :], rq_b)
                yreg = qT_sb[:, :, ts]
                q3d = bass.AP(tensor=yreg.tensor, offset=yreg.offset,
                              ap=[yreg.ap[0], [T, NHEAD], [1, 128]])
                nc.sync.dma_start_transpose(out=q3d, in_=q_fin)
            kreg = kT_sb[:, g * 512:(g + 1) * 512]
            k3d = bass.AP(tensor=kreg.tensor, offset=kreg.offset,
                          ap=[kreg.ap[0], [128, 4], [1, 128]])
            nc.sync.dma_start_transpose(out=k3d, in_=kfin_g)

            # ============== Phase B: attention for q chunk g ==============
            qc = g
            nki = 4 * qc + 4
            # units: pairs of full-width ki tiles, then the 4 diagonal tiles
            units = [(2 * j, 2 * j + 1) for j in range(2 * qc)] \
                + [(ki,) for ki in range(4 * qc, nki)]
            for h in range(NHEAD):
                y01 = ps.tile([128, 2, HD + 1], F32, tag="y01", bufs=1)
                y23 = ps.tile([128, 2, HD + 1], F32, tag="y23", bufs=1)
                prev = None
                for unit in units:
                    s2 = ps.tile([128, 2, 512], F32, tag="s", bufs=2)
                    p2 = b_sb.tile([128, 2, 512], BF16, tag="p")
                    if len(unit) == 2:
                        for sl, ki in enumerate(unit):
                            nc.tensor.matmul(
                                s2[:, sl, :],
                                lhsT=kT_sb[:, ki * 128:(ki + 1) * 128],
                                rhs=qT_sb[:, h, qc * 512:(qc + 1) * 512],
                                start=True, stop=True)
                        nc.scalar.activation(
                            out=p2.rearrange("p a b -> p (a b)"),
                            in_=s2.rearrange("p a b -> p (a b)"),
                            func=AF.Exp, scale=SM_SCALE)
                    else:
                        ki = unit[0]
                        m = ki - 4 * qc
                        nq = 512 - 128 * m
                        q_lo = qc * 512 + 128 * m
                        nc.tensor.matmul(
                            s2[:, 0, 0:nq],
                            lhsT=kT_sb[:, ki * 128:(ki + 1) * 128],
                            rhs=qT_sb[:, h, q_lo:(qc + 1) * 512],
                            start=True, stop=True)
                        nc.scalar.activation(out=p2[:, 0, 0:nq],
                                             in_=s2[:, 0, 0:nq],
                                             func=AF.Exp, scale=SM_SCALE)
                        nc.gpsimd.tensor_mul(p2[:, 0, 0:128],
                                             p2[:, 0, 0:128], mask_sb)
                    if prev is not None:
                        _issue_pv(nc, prev, y01, y23, v_sb, qc)
                    drain(proj_gen, 3)
                    prev = (unit, p2)
                _issue_pv(nc, prev, y01, y23, v_sb, qc)

                # normalize + gate -> bf16 staging, transpose on scalar queue
                rd4 = b_sb.tile([128, 4], F32, tag="rd4")
                nc.vector.reciprocal(rd4[:, 0:2], bass.AP(
                    tensor=y01.tensor, offset=y01.offset + HD,
                    ap=[y01.ap[0], [HD + 1, 2]]))
                nc.vector.reciprocal(rd4[:, 2:4], bass.AP(
                    tensor=y23.tensor, offset=y23.offset + HD,
                    ap=[y23.ap[0], [HD + 1, 2]]))
                sc4 = b_sb.tile([128, 4], F32, tag="sc4")
                gslice = bass.AP(
                    tensor=gate_sb.tensor,
                    offset=gate_sb.offset + (4 * qc) * NHEAD + h,
                    ap=[gate_sb.ap[0], [NHEAD, 4], [1, 1]])
                nc.vector.tensor_mul(sc4, rd4, gslice)
                y_stage = b_sb.tile([128, 4, HD], BF16, tag="y_stage", bufs=2)
                for qs in range(4):
                    ytile = y01 if qs < 2 else y23
                    nc.scalar.activation(out=y_stage[:, qs, :],
                                         in_=ytile[:, qs % 2, 0:HD],
                                         func=AF.Copy,
                                         scale=sc4[:, qs:qs + 1])
                yreg = yT_sb[:, h, qc * 512:(qc + 1) * 512]
                y3d = bass.AP(tensor=yreg.tensor, offset=yreg.offset,
                              ap=[yreg.ap[0], [128, 4], [1, 128]])
                nc.sync.dma_start_transpose(out=y3d, in_=y_stage)

            drain(proj_gen, 10000)
            proj_gen = make_proj(qc)

        drain(proj_gen, 10000)

    nc.compile()
    return nc


def _issue_pv(nc, prev, y01, y23, v_sb, qc):
    unit, p2 = prev
    for sl, ki in enumerate(unit):
        m = ki - 4 * qc
        for qs in range(max(m, 0), 4):
            ytile = y01 if qs < 2 else y23
            pcol = (qs - max(m, 0)) * 128
            nc.tensor.matmul(
                ytile[:, qs % 2, :],
                lhsT=p2[:, sl, pcol:pcol + 128],
                rhs=v_sb[:, ki, :],
                start=(ki == 0 and qs % 2 == 0),
                stop=(ki == 4 * qc + qs and qs % 2 == 1))


def _get_program():
    if "nc" not in _CACHE:
        _CACHE["nc"] = _build_program()
    return _CACHE["nc"]


def _host_prep(x, Wq, Wk, Wv, Wproj, q_gain, gate_w, gate_b):
    """Build the 8 per-core input maps."""
    f = np.float32
    x = np.asarray(x, f)
    WqT = np.asarray(Wq, f).T.astype(NPBF)       # [D, 2048]
    WkT = np.asarray(Wk, f).T.astype(NPBF)       # [D, 512]
    WvT = np.asarray(Wv, f).T.astype(NPBF)
    WpT = np.ascontiguousarray(np.asarray(Wproj, f).T.astype(NPBF))  # [D, D]
    gwT = np.asarray(gate_w, f).T.astype(NPBF)   # [D, 16]
    q_gain = np.asarray(q_gain, f)
    gate_b = np.asarray(gate_b, f)

    inv_freq = 1.0 / (ROPE_BASE ** (np.arange(0, HD, 2, dtype=f) / HD))
    tpos = np.arange(T, dtype=f)
    freqs = np.outer(tpos, inv_freq)             # [T, 64]
    cosF = np.concatenate([np.cos(freqs), np.cos(freqs)], axis=1)
    sinF = np.concatenate([np.sin(freqs), -np.sin(freqs)], axis=1)
    cosF = cosF.astype(NPBF)                     # [T, 128]
    sinF = sinF.astype(NPBF)

    kloc = np.arange(128)[:, None]
    qloc = np.arange(128)[None, :]
    mask = (qloc >= kloc).astype(NPBF)           # [128, 128]

    xT = [np.ascontiguousarray(x[b].T).astype(NPBF) for b in range(B)]

    in_maps = []
    for core in range(8):
        b, g = divmod(core, 4)
        wkvg = np.concatenate([
            WkT[:, 128 * g:128 * (g + 1)],
            WvT[:, 128 * g:128 * (g + 1)],
            gwT[:, NHEAD * g:NHEAD * (g + 1)],
        ], axis=1)                               # [D, 260]
        in_maps.append({
            "xT": xT[b],
            "wq": np.ascontiguousarray(WqT[:, 512 * g:512 * (g + 1)]),
            "wkvg": np.ascontiguousarray(wkvg),
            "wproj": np.ascontiguousarray(WpT[512 * g:512 * (g + 1), :]),
            "cosd": cosF,
            "sind": sinF,
            "qgain": np.ascontiguousarray(q_gain[NHEAD * g:NHEAD * (g + 1)][None, :]),
            "gateb": np.ascontiguousarray(gate_b[NHEAD * g:NHEAD * (g + 1)][None, :]),
            "masks": mask,
        })
    return in_maps


def kernel(**inputs):
    nc = _get_program()
    in_maps = _host_prep(**inputs)
    res = run_bass_kernel_spmd(nc, in_maps, list(range(8)))
    parts = [r["out"] for r in res.results]
    out = np.empty((B, T, D), np.float32)
    for b in range(B):
        out[b] = (parts[4 * b].astype(np.float32)
                  + parts[4 * b + 1].astype(np.float32)
                  + parts[4 * b + 2].astype(np.float32)
                  + parts[4 * b + 3].astype(np.float32))
    return out


# revision 20
# speedup vs baseline: 1.0092x; 1.0092x over previous
"""Trainium2 Bass kernel for CausalSelfAttention (GQA + qk-rmsnorm + rope + head gating).

Sharding: 8 cores = 2 (batch) x 4 (kv-head groups). Each core computes the
full attention for one batch element and one kv-head group (4 q heads), plus
its slice of the output projection; partial projection outputs are summed on
the host (bf16 partials, fp32 accumulation).

Per-core pipeline, merged across phases per 4-token-tile group g:
  A) fused QKV+gate projection for tiles 4g..4g+3 -> bf16 cast ->
     full-width rope (3 DVE ops/group, swap-AP + pre-signed sin table) ->
     rms stats (mul+reduce) -> scale -> DMA-transpose q,k to head-dim-major
  B) causal attention for q chunk g in S^T layout, software-pipelined:
     S^T = K @ Q^T (PE), P = exp(S/sqrt(d)) (ACT), diagonal mask (DVE),
     Y = P @ [V | 1] (PE, ones column = softmax denominator), with the
     previous chunk's output-projection matmuls interleaved into the
     tensor queue to fill exp-latency stalls
  C) output projection partial in bf16, 256KB stores
k-side elementwise work (rope/stats/scale/v-copy) runs on GpSimd to keep
the DVE under the tensor-engine roofline.
"""

import numpy as np
import ml_dtypes
from contextlib import ExitStack

import concourse.bass as bass
import concourse.bacc as bacc
import concourse.mybir as mybir
import concourse.tile as tile
from concourse.bass_utils import run_bass_kernel_spmd

BF16 = mybir.dt.bfloat16
F32 = mybir.dt.float32
NPBF = ml_dtypes.bfloat16

B, T, D = 2, 2048, 2048
H, HKV, HD = 16, 4, 128
HALF = HD // 2
NHEAD = H // HKV          # q heads per core (group)
NT = T // 128             # 16 token tiles
NG = 4                    # 4-tile groups
NCHUNK = D // 128         # 16 contraction chunks
NKVG = HD + HD + NHEAD    # 128 k + 128 v + 4 gate = 260
ROPE_BASE = 10000.0
EPS = float(np.finfo(np.float32).eps)
SM_SCALE = 1.0 / float(np.sqrt(HD))

_CACHE = {}


def _build_program():
    nc = bacc.Bacc("TRN2", target_bir_lowering=False, debug=False,
                   enable_asserts=False, num_devices=8)

    xT_d = nc.dram_tensor("xT", [D, T], BF16, kind="ExternalInput").ap()
    wq_d = nc.dram_tensor("wq", [D, NHEAD * HD], BF16, kind="ExternalInput").ap()
    wkvg_d = nc.dram_tensor("wkvg", [D, NKVG], BF16, kind="ExternalInput").ap()
    wproj_d = nc.dram_tensor("wproj", [NHEAD * HD, D], BF16, kind="ExternalInput").ap()
    cos_d = nc.dram_tensor("cosd", [T, HD], BF16, kind="ExternalInput").ap()
    sin_d = nc.dram_tensor("sind", [T, HD], BF16, kind="ExternalInput").ap()
    qgain_d = nc.dram_tensor("qgain", [1, NHEAD], F32, kind="ExternalInput").ap()
    gateb_d = nc.dram_tensor("gateb", [1, NHEAD], F32, kind="ExternalInput").ap()
    mask_d = nc.dram_tensor("masks", [128, 128], BF16, kind="ExternalInput").ap()
    out_d = nc.dram_tensor("out", [T, D], BF16, kind="ExternalOutput").ap()

    AF = mybir.ActivationFunctionType

    with tile.TileContext(nc) as tc, ExitStack() as ctx:
        consts = ctx.enter_context(tc.tile_pool(name="consts", bufs=1))

        # ---- resident tensors (loads interleaved so compute starts early) ----
        xT_sb = consts.tile([128, NCHUNK, T], BF16)
        wq_sb = consts.tile([128, NCHUNK, NHEAD * HD], BF16)
        wkvg_sb = consts.tile([128, NCHUNK, NKVG], BF16)
        for c in range(NCHUNK):
            cs = slice(c * 128, (c + 1) * 128)
            nc.sync.dma_start(out=xT_sb[:, c, :], in_=xT_d[cs, :])
            nc.sync.dma_start(out=wq_sb[:, c, :], in_=wq_d[cs, :])
            nc.sync.dma_start(out=wkvg_sb[:, c, :], in_=wkvg_d[cs, :])
        wproj_sb = consts.tile([128, NHEAD, D], BF16)
        for h in range(NHEAD):
            nc.scalar.dma_start(out=wproj_sb[:, h, :],
                                in_=wproj_d[h * 128:(h + 1) * 128, :])
        cos_sb = consts.tile([128, NT, HD], BF16)
        nc.scalar.dma_start(out=cos_sb,
                            in_=cos_d.rearrange("(tt p) i -> p tt i", p=128))
        sin_sb = consts.tile([128, NT, HD], BF16)
        nc.scalar.dma_start(out=sin_sb,
                            in_=sin_d.rearrange("(tt p) i -> p tt i", p=128))
        qgain_sb = consts.tile([128, NHEAD], F32)
        nc.scalar.dma_start(out=qgain_sb, in_=bass.AP(
            tensor=qgain_d.tensor, offset=qgain_d.offset,
            ap=[[0, 128], [1, NHEAD]]))
        gateb_sb = consts.tile([128, NHEAD], F32)
        nc.scalar.dma_start(out=gateb_sb, in_=bass.AP(
            tensor=gateb_d.tensor, offset=gateb_d.offset,
            ap=[[0, 128], [1, NHEAD]]))
        mask_sb = consts.tile([128, 128], BF16)
        nc.scalar.dma_start(out=mask_sb, in_=mask_d)

        qT_sb = consts.tile([128, NHEAD, T], BF16)   # head-dim-major q
        kT_sb = consts.tile([128, T], BF16)          # head-dim-major k
        v_sb = consts.tile([128, NT, HD + 1], BF16)  # [v | ones] per ki tile
        nc.vector.memset(v_sb[:, :, HD:HD + 1], 1.0)
        yT_sb = consts.tile([128, NHEAD, T], BF16)   # head-dim-major gated y
        gate_sb = consts.tile([128, NT, NHEAD], F32)
        eps_sb = consts.tile([128, 1], F32)
        nc.vector.memset(eps_sb, EPS)

        a_sb = ctx.enter_context(tc.tile_pool(name="phA", bufs=2))
        b_sb = ctx.enter_context(tc.tile_pool(name="phB", bufs=3))
        ps = ctx.enter_context(tc.tile_pool(name="ps", bufs=1, space="PSUM"))

        def swap_halves(ap3):
            """[p, n, HD] AP -> same with the two HD/2 halves swapped."""
            return bass.AP(tensor=ap3.tensor, offset=ap3.offset + HALF,
                           ap=[ap3.ap[0], ap3.ap[1], [-HALF, 2], [1, HALF]])

        def split_halves(ap3):
            """[p, n, HD] AP -> [p, n, 2, HD/2] (no swap), to match shapes."""
            return bass.AP(tensor=ap3.tensor, offset=ap3.offset,
                           ap=[ap3.ap[0], ap3.ap[1], [HALF, 2], [1, HALF]])

        # ---------------- proj thunk generator (phase C) ----------------
        def make_proj(qc):
            def gen():
                for half in range(2):          # nch pairs (0,1) and (2,3)
                    for qs in range(4):
                        tt = qc * 4 + qs
                        ts = slice(tt * 128, (tt + 1) * 128)
                        o_st = b_sb.tile([128, 1024], BF16, tag="o_st", bufs=2)
                        for sub in range(2):
                            nch = half * 2 + sub
                            o_ps = ps.tile([128, 512], F32, tag="qkv",
                                           bufs=2)
                            for h in range(NHEAD):
                                yield lambda o_ps=o_ps, h=h, ts=ts, nch=nch: \
                                    nc.tensor.matmul(
                                        o_ps, lhsT=yT_sb[:, h, ts],
                                        rhs=wproj_sb[:, h,
                                                     nch * 512:(nch + 1) * 512],
                                        start=(h == 0), stop=(h == NHEAD - 1))
                            yield lambda o_ps=o_ps, o_st=o_st, sub=sub: \
                                nc.vector.tensor_copy(
                                    out=o_st[:, sub * 512:(sub + 1) * 512],
                                    in_=o_ps)
                        yield lambda o_st=o_st, ts=ts, half=half: \
                            nc.sync.dma_start(
                                out=out_d[ts, half * 1024:(half + 1) * 1024],
                                in_=o_st)
            return gen()

        def drain(gen, n):
            if gen is None:
                return
            for _ in range(n):
                try:
                    next(gen)()
                except StopIteration:
                    return

        proj_gen = None

        for g in range(NG):
            # ================= Phase A: tiles 4g .. 4g+3 =================
            qa_g = a_sb.tile([128, 4, NHEAD * HD], BF16, tag="qa_g")
            kb_g = a_sb.tile([128, 4, NKVG], BF16, tag="kb_g")
            qst_g = a_sb.tile([128, 4, NHEAD, HD], BF16, tag="qst_g")
            kst_g = a_sb.tile([128, 4, HD], BF16, tag="kst_g")
            uk_g = a_sb.tile([128, 4, HD], BF16, tag="uk_g")
            sq_g = a_sb.tile([128, 4, NHEAD * HD], BF16, tag="sq_g")
            sqk_g = a_sb.tile([128, 4, HD], BF16, tag="sqk_g")
            glog_g = a_sb.tile([128, 4, NHEAD], F32, tag="glog_g")
            msq_g = a_sb.tile([128, 4, NHEAD + 1], F32, tag="msq_g")
            rq_g = a_sb.tile([128, 4, NHEAD], F32, tag="rq_g")

            for ti in range(4):
                tt = g * 4 + ti
                ts = slice(tt * 128, (tt + 1) * 128)
                q_ps = ps.tile([128, 512], F32, tag="qkv", bufs=2)
                for c in range(NCHUNK):
                    nc.tensor.matmul(q_ps, lhsT=xT_sb[:, c, ts],
                                     rhs=wq_sb[:, c, :],
                                     start=(c == 0), stop=(c == NCHUNK - 1))
                nc.vector.tensor_copy(out=qa_g[:, ti, :], in_=q_ps)
                b_ps = ps.tile([128, 512], F32, tag="qkv", bufs=2)
                for c in range(NCHUNK):
                    nc.tensor.matmul(b_ps[:, 0:NKVG], lhsT=xT_sb[:, c, ts],
                                     rhs=wkvg_sb[:, c, :],
                                     start=(c == 0), stop=(c == NCHUNK - 1))
                nc.vector.tensor_copy(out=kb_g[:, ti, :], in_=b_ps[:, 0:NKVG])
                nc.vector.tensor_copy(out=v_sb[:, tt, 0:HD],
                                      in_=kb_g[:, ti, HD:2 * HD])

            # gate logits (+bias broadcast over the 4 tiles)
            gateb_b = bass.AP(tensor=gateb_sb.tensor, offset=gateb_sb.offset,
                              ap=[gateb_sb.ap[0], [0, 4], [1, NHEAD]])
            nc.gpsimd.tensor_add(glog_g, kb_g[:, :, 2 * HD:2 * HD + NHEAD],
                                 gateb_b)

            # ---- q rope per tile: full-width pre-signed tables, h-bcast ----
            for ti in range(4):
                tt = g * 4 + ti
                u_t = a_sb.tile([128, NHEAD, HD], BF16, tag="u_t", bufs=2)
                qa_t = qa_g[:, ti, :]
                qa_h = bass.AP(tensor=qa_t.tensor, offset=qa_t.offset,
                               ap=[qa_t.ap[0], [HD, NHEAD], [1, HD]])
                qst_t = qst_g[:, ti, :, :]
                cos_t = cos_sb[:, tt, :]
                cos_h = bass.AP(tensor=cos_t.tensor, offset=cos_t.offset,
                                ap=[cos_t.ap[0], [0, NHEAD], [1, HD]])
                sin_t = sin_sb[:, tt, :]
                sin_h = bass.AP(tensor=sin_t.tensor, offset=sin_t.offset,
                                ap=[sin_t.ap[0], [0, NHEAD], [1, HD]])
                nc.vector.tensor_mul(qst_t, qa_h, cos_h)
                nc.vector.tensor_mul(split_halves(u_t), swap_halves(qa_h),
                                     split_halves(sin_h))
                nc.vector.tensor_add(qst_t, qst_t, u_t)

            # ---- k rope on gpsimd ----
            cos_t = cos_sb[:, g * 4:(g + 1) * 4, :]
            sin_t = sin_sb[:, g * 4:(g + 1) * 4, :]
            kin = kb_g[:, :, 0:HD]
            nc.gpsimd.tensor_mul(kst_g, kin, cos_t)
            nc.gpsimd.tensor_mul(split_halves(uk_g), swap_halves(kin),
                                 split_halves(sin_t))
            nc.gpsimd.tensor_add(kst_g, kst_g, uk_g)

            # ---- mean-square (rope preserves norms; use rotated values) ----
            nc.vector.tensor_mul(sq_g, qst_g, qst_g)
            msq_q = bass.AP(tensor=msq_g.tensor, offset=msq_g.offset,
                            ap=[msq_g.ap[0], [NHEAD + 1, 4], [1, NHEAD]])
            sq_red = bass.AP(tensor=sq_g.tensor, offset=sq_g.offset,
                             ap=[sq_g.ap[0], [HD, 16], [1, HD]])
            nc.vector.tensor_reduce(msq_q, sq_red,
                                    axis=mybir.AxisListType.X,
                                    op=mybir.AluOpType.add)
            nc.gpsimd.tensor_mul(sqk_g, kst_g, kst_g)
            nc.vector.tensor_reduce(msq_g[:, :, NHEAD:NHEAD + 1], sqk_g,
                                    axis=mybir.AxisListType.X,
                                    op=mybir.AluOpType.add)

            # ---- batched scalar math (Exp+Tanh: one ACT table set) ----
            # gate = 0.5 + 0.5*tanh(glog/2)
            gt_g = a_sb.tile([128, 4, NHEAD], F32, tag="gt_g")
            nc.scalar.activation(
                out=gt_g.rearrange("p a b -> p (a b)"),
                in_=glog_g.rearrange("p a b -> p (a b)"), func=AF.Tanh,
                scale=0.5)
            nc.vector.tensor_scalar(gate_sb[:, g * 4:(g + 1) * 4, :], gt_g,
                                    0.5, 0.5, op0=mybir.AluOpType.mult,
                                    op1=mybir.AluOpType.add)
            # rinv = 1/sqrt(ms), ms = msq/HD + eps, via 3 Newton steps on DVE
            # from r0 = 2.2/(1+ms)
            ms_g = a_sb.tile([128, 4, NHEAD + 1], F32, tag="ms_g")
            nc.vector.tensor_scalar(ms_g, msq_g, 1.0 / HD, EPS,
                                    op0=mybir.AluOpType.mult,
                                    op1=mybir.AluOpType.add)
            a1_g = a_sb.tile([128, 4, NHEAD + 1], F32, tag="a1_g")
            nc.vector.tensor_scalar_add(a1_g, ms_g, 1.0)
            rbuf = [a_sb.tile([128, 4, NHEAD + 1], F32, tag=f"r{i}_g",
                              name=f"r{i}_g") for i in range(4)]
            t_g = a_sb.tile([128, 4, NHEAD + 1], F32, tag="t_g")
            u_g = a_sb.tile([128, 4, NHEAD + 1], F32, tag="u_g")
            w_g = a_sb.tile([128, 4, NHEAD + 1], F32, tag="w_g")
            nc.vector.reciprocal(rbuf[0], a1_g)
            consts_nw = [(-0.5 * 2.2 ** 3, 1.5 * 2.2), (-0.5, 1.5),
                         (-0.5, 1.5)]
            for it, (cm, ca) in enumerate(consts_nw):
                r_in, r_out = rbuf[it], rbuf[it + 1]
                nc.vector.tensor_mul(t_g, r_in, r_in)
                nc.vector.tensor_mul(u_g, ms_g, t_g)
                nc.vector.tensor_scalar(w_g, u_g, cm, ca,
                                        op0=mybir.AluOpType.mult,
                                        op1=mybir.AluOpType.add)
                nc.vector.tensor_mul(r_out, r_in, w_g)
            rinv_g = rbuf[3]
            qgain_b = bass.AP(tensor=qgain_sb.tensor, offset=qgain_sb.offset,
                              ap=[qgain_sb.ap[0], [0, 4], [1, NHEAD]])
            nc.vector.tensor_mul(rq_g, rinv_g[:, :, 0:NHEAD], qgain_b)

            # ---- scale into fresh staging tiles + transpose ----
            kfin_g = a_sb.tile([128, 4, HD], BF16, tag="kfin_g")
            rk_b = bass.AP(tensor=rinv_g.tensor,
                           offset=rinv_g.offset + NHEAD,
                           ap=[rinv_g.ap[0], [NHEAD + 1, 4], [0, HD]])
            nc.vector.tensor_mul(kfin_g, kst_g, rk_b)
            for ti in range(4):
                tt = g * 4 + ti
                ts = slice(tt * 128, (tt + 1) * 128)
                q_fin = a_sb.tile([128, NHEAD, HD], BF16, tag="q_fin",
                                  bufs=2)
                rq_t = rq_g[:, ti, :]
                rq_b = bass.AP(tensor=rq_t.tensor, offset=rq_t.offset,
                               ap=[rq_t.ap[0], [1, NHEAD], [0, HD]])
                nc.vector.tensor_mul(q_fin, qst_g[:, ti, :, :], rq_b)
                yreg = qT_sb[:, :, ts]
                q3d = bass.AP(tensor=yreg.tensor, offset=yreg.offset,
                              ap=[yreg.ap[0], [T, NHEAD], [1, 128]])
                nc.sync.dma_start_transpose(out=q3d, in_=q_fin)
            kreg = kT_sb[:, g * 512:(g + 1) * 512]
            k3d = bass.AP(tensor=kreg.tensor, offset=kreg.offset,
                          ap=[kreg.ap[0], [128, 4], [1, 128]])
            nc.sync.dma_start_transpose(out=k3d, in_=kfin_g)

            # ============== Phase B: attention for q chunk g ==============
            qc = g
            nki = 4 * qc + 4
            # units: pairs of full-width ki tiles, then the 4 diagonal tiles
            units = [(2 * j, 2 * j + 1) for j in range(2 * qc)] \
                + [(ki,) for ki in range(4 * qc, nki)]
            for h in range(NHEAD):
                y01 = ps.tile([128, 2, HD + 1], F32, tag="y01", bufs=1)
                y23 = ps.tile([128, 2, HD + 1], F32, tag="y23", bufs=1)
                prev = None
                for unit in units:
                    s2 = ps.tile([128, 2, 512], F32, tag="s", bufs=2)
                    p2 = b_sb.tile([128, 2, 512], BF16, tag="p")
                    if len(unit) == 2:
                        for sl, ki in enumerate(unit):
                            nc.tensor.matmul(
                                s2[:, sl, :],
                                lhsT=kT_sb[:, ki * 128:(ki + 1) * 128],
                                rhs=qT_sb[:, h, qc * 512:(qc + 1) * 512],
                                start=True, stop=True)
                        nc.scalar.activation(
                            out=p2.rearrange("p a b -> p (a b)"),
                            in_=s2.rearrange("p a b -> p (a b)"),
                            func=AF.Exp, scale=SM_SCALE)
                    else:
                        ki = unit[0]
                        m = ki - 4 * qc
                        nq = 512 - 128 * m
                        q_lo = qc * 512 + 128 * m
                        nc.tensor.matmul(
                            s2[:, 0, 0:nq],
                            lhsT=kT_sb[:, ki * 128:(ki + 1) * 128],
                            rhs=qT_sb[:, h, q_lo:(qc + 1) * 512],
                            start=True, stop=True)
                        nc.scalar.activation(out=p2[:, 0, 0:nq],
                                             in_=s2[:, 0, 0:nq],
                                             func=AF.Exp, scale=SM_SCALE)
                        nc.gpsimd.tensor_mul(p2[:, 0, 0:128],
                                             p2[:, 0, 0:128], mask_sb)
                    if prev is not None:
                        _issue_pv(nc, prev, y01, y23, v_sb, qc)
                    drain(proj_gen, 3)
                    prev = (unit, p2)
                _issue_pv(nc, prev, y01, y23, v_sb, qc)

                # normalize + gate -> bf16 staging, transpose on scalar queue
                rd4 = b_sb.tile([128, 4], F32, tag="rd4")
                nc.vector.reciprocal(rd4[:, 0:2], bass.AP(
                    tensor=y01.tensor, offset=y01.offset + HD,
                    ap=[y01.ap[0], [HD + 1, 2]]))
                nc.vector.reciprocal(rd4[:, 2:4], bass.AP(
                    tensor=y23.tensor, offset=y23.offset + HD,
                    ap=[y23.ap[0], [HD + 1, 2]]))
                sc4 = b_sb.tile([128, 4], F32, tag="sc4")
                gslice = bass.AP(
                    tensor=gate_sb.tensor,
                    offset=gate_sb.offset + (4 * qc) * NHEAD + h,
                    ap=[gate_sb.ap[0], [NHEAD, 4], [1, 1]])
                nc.vector.tensor_mul(sc4, rd4, gslice)
                y_stage = b_sb.tile([128, 4, HD], BF16, tag="y_stage", bufs=2)
                for qs in range(4):
                    ytile = y01 if qs < 2 else y23
                    nc.scalar.activation(out=y_stage[:, qs, :],
                                         in_=ytile[:, qs % 2, 0:HD],
                                         func=AF.Copy,
                                         scale=sc4[:, qs:qs + 1])
                yreg = yT_sb[:, h, qc * 512:(qc + 1) * 512]
                y3d = bass.AP(tensor=yreg.tensor, offset=yreg.offset,
                              ap=[yreg.ap[0], [128, 4], [1, 128]])
                nc.sync.dma_start_transpose(out=y3d, in_=y_stage)

            drain(proj_gen, 10000)
            proj_gen = make_proj(qc)

        drain(proj_gen, 10000)

    nc.compile()
    return nc


def _issue_pv(nc, prev, y01, y23, v_sb, qc):
    unit, p2 = prev
    for sl, ki in enumerate(unit):
        m = ki - 4 * qc
        for qs in range(max(m, 0), 4):
            ytile = y01 if qs < 2 else y23
            pcol = (qs - max(m, 0)) * 128
            nc.tensor.matmul(
                ytile[:, qs % 2, :],
                lhsT=p2[:, sl, pcol:pcol + 128],
                rhs=v_sb[:, ki, :],
                start=(ki == 0 and qs % 2 == 0),
                stop=(ki == 4 * qc + qs and qs % 2 == 1))


def _get_program():
    if "nc" not in _CACHE:
        _CACHE["nc"] = _build_program()
    return _CACHE["nc"]


def _host_prep(x, Wq, Wk, Wv, Wproj, q_gain, gate_w, gate_b):
    """Build the 8 per-core input maps."""
    f = np.float32
    x = np.asarray(x, f)
    WqT = np.asarray(Wq, f).T.astype(NPBF)       # [D, 2048]
    WkT = np.asarray(Wk, f).T.astype(NPBF)       # [D, 512]
    WvT = np.asarray(Wv, f).T.astype(NPBF)
    WpT = np.ascontiguousarray(np.asarray(Wproj, f).T.astype(NPBF))  # [D, D]
    gwT = np.asarray(gate_w, f).T.astype(NPBF)   # [D, 16]
    q_gain = np.asarray(q_gain, f)
    gate_b = np.asarray(gate_b, f)

    inv_freq = 1.0 / (ROPE_BASE ** (np.arange(0, HD, 2, dtype=f) / HD))
    tpos = np.arange(T, dtype=f)
    freqs = np.outer(tpos, inv_freq)             # [T, 64]
    cosF = np.concatenate([np.cos(freqs), np.cos(freqs)], axis=1)
    sinF = np.concatenate([np.sin(freqs), -np.sin(freqs)], axis=1)
    cosF = cosF.astype(NPBF)                     # [T, 128]
    sinF = sinF.astype(NPBF)

    kloc = np.arange(128)[:, None]
    qloc = np.arange(128)[None, :]
    mask = (qloc >= kloc).astype(NPBF)           # [128, 128]

    xT = [np.ascontiguousarray(x[b].T).astype(NPBF) for b in range(B)]

    in_maps = []
    for core in range(8):
        b, g = divmod(core, 4)
        wkvg = np.concatenate([
            WkT[:, 128 * g:128 * (g + 1)],
            WvT[:, 128 * g:128 * (g + 1)],
            gwT[:, NHEAD * g:NHEAD * (g + 1)],
        ], axis=1)                               # [D, 260]
        in_maps.append({
            "xT": xT[b],
            "wq": np.ascontiguousarray(WqT[:, 512 * g:512 * (g + 1)]),
            "wkvg": np.ascontiguousarray(wkvg),
            "wproj": np.ascontiguousarray(WpT[512 * g:512 * (g + 1), :]),
            "cosd": cosF,
            "sind": sinF,
            "qgain": np.ascontiguousarray(q_gain[NHEAD * g:NHEAD * (g + 1)][None, :]),
            "gateb": np.ascontiguousarray(gate_b[NHEAD * g:NHEAD * (g + 1)][None, :]),
            "masks": mask,
        })
    return in_maps


def kernel(**inputs):
    nc = _get_program()
    in_maps = _host_prep(**inputs)
    res = run_bass_kernel_spmd(nc, in_maps, list(range(8)))
    parts = [r["out"] for r in res.results]
    out = np.empty((B, T, D), np.float32)
    for b in range(B):
        out[b] = (parts[4 * b].astype(np.float32)
                  + parts[4 * b + 1].astype(np.float32)
                  + parts[4 * b + 2].astype(np.float32)
                  + parts[4 * b + 3].astype(np.float32))
    return out
